# revision 22
# baseline (speedup 1.0000x reference)
"""MoE gate routing kernel for 8 Trainium2 NeuronCores.

Problem: hidden_states [4, 8192, 2048] f32, weight [8, 2048] f32.
  logits = x @ w.T ; scores = softmax(logits); top-2 (values+indices);
  normalized top-2 weights; seq-aux load-balancing loss scalar.

Sharding: 4096 tokens per core (token-parallel; each core's tokens lie in
one batch row). The gate weight is replicated. Per-core partial per-expert
score sums and top-2 counts are folded into the aux-loss scalar on host.

The device kernel consumes x pre-transposed ([H, T_core]) so the H
(contraction) dim lands on SBUF partitions with unit-efficiency DMAs.
All matmuls are exact fp32.
"""
import os
import numpy as np

B, S, H = 4, 8192, 2048
E, K = 8, 2
NCORES = 8
T = B * S                 # 32768 tokens
TC = T // NCORES          # 4096 tokens per core
NQ = 4                    # quarters per core
TQ = TC // NQ             # 1024 tokens per quarter
NG = 2                    # 512-token groups per quarter
TG = TQ // NG             # 512
NCHUNK = H // 128         # 16 contraction chunks
ALPHA = 0.1

_CACHE = {}
LAST_EXEC_NS = None
LAST_PROFILE = None


def _build(variant, loop_n=None):
    import contextlib
    import concourse.bacc as bacc
    import concourse.tile as tile
    from concourse import mybir

    F32 = mybir.dt.float32
    I32 = mybir.dt.int32
    U32 = mybir.dt.uint32
    AF = mybir.ActivationFunctionType
    ALU = mybir.AluOpType
    AX = mybir.AxisListType

    nc = bacc.Bacc("TRN2", target_bir_lowering=False, debug=False,
                   num_devices=NCORES)
    if variant == "v5":
        xT = nc.declare_dram_parameter("xT", [2 * H, TC // 2], F32,
                                       isOutput=False)
    else:
        xT = nc.declare_dram_parameter("xT", [H, TC], F32, isOutput=False)
    wT = nc.declare_dram_parameter("wT", [H, E], F32, isOutput=False)
    eye = nc.declare_dram_parameter("eye", [E, E], F32, isOutput=False)
    idx_out = nc.declare_dram_parameter("idx_out", [128, TC * K // 128], I32,
                                        isOutput=True)
    w_out = nc.declare_dram_parameter("w_out", [128, TC * K // 128], F32,
                                      isOutput=True)
    stats_out = nc.declare_dram_parameter("stats_out", [2 * E * (TG // 128), 1],
                                          F32, isOutput=True)

    NSUB = TG // 128            # 128-token subtiles per group = 4
    NGRP = NQ * NG              # 8 groups of 512 tokens per core

    with tile.TileContext(nc) as tc:
        with (
            tc.tile_pool(name="const", bufs=1) as cpool,
            tc.tile_pool(name="xt", bufs=12 if variant in ("v6", "v7", "v8", "v9") else (8 if variant in ("v4", "v5") else 6)) as xpool,
            tc.tile_pool(name="lg", bufs=4) as lgpool,
            tc.tile_pool(name="lt", bufs=2) as ltpool,
            tc.tile_pool(name="small", bufs=2) as spool,
            tc.tile_pool(name="outacc", bufs=1) as opool,
            tc.tile_pool(name="psL", bufs=4, space="PSUM") as psL,
            tc.tile_pool(name="psT", bufs=2, space="PSUM") as psT,
            tc.tile_pool(name="psS", bufs=1, space="PSUM") as psS,
        ):
            # --- constants ---
            wt_sb = cpool.tile([128, NCHUNK, E], F32)     # weight chunks
            for c in range(NCHUNK):
                nc.sync.dma_start(out=wt_sb[:, c, :],
                                  in_=wT[128 * c:128 * (c + 1), :])
            eye_sb = cpool.tile([E, E], F32)
            nc.sync.dma_start(out=eye_sb[:], in_=eye[:])
            ones_sb = cpool.tile([128, 1], F32)
            nc.vector.memset(ones_sb[:], 1.0)

            # --- persistent accumulators ---
            idx_all = opool.tile([128, NGRP, NSUB, K], I32)
            w_all = opool.tile([128, NGRP, NSUB, K], F32)
            stat_ps = psS.tile([2 * E * NSUB, 1], F32)

            # bench-only: repeat the whole body on-device to time the
            # steady-state kernel without per-execution dispatch overhead
            loop_cm = tc.For_i(0, loop_n, 1) if loop_n else contextlib.nullcontext()
            loop_cm.__enter__()

            if variant == "dma":
                # memory-floor probe: stream all of xT in, minimal consume
                acc = cpool.tile([128, 1], F32)
                for q in range(2):
                    for c in range(NCHUNK):
                        xt = xpool.tile([128, TC // 2], F32, tag="xt")
                        nc.sync.dma_start(
                            out=xt[:],
                            in_=xT[128 * c:128 * (c + 1),
                                   (TC // 2) * q:(TC // 2) * (q + 1)])
                        nc.vector.tensor_reduce(acc[:], xt[:, 0:8], AX.X,
                                                ALU.add)
                loop_cm.__exit__(None, None, None)
                nc.vector.memset(idx_all[:].rearrange("p a b c -> p (a b c)"), 0)
                nc.vector.memset(w_all[:].rearrange("p a b c -> p (a b c)"), 0.0)
                stat_sb0 = cpool.tile([2 * E * NSUB, 1], F32)
                nc.vector.memset(stat_sb0[:], 0.0)
                nc.sync.dma_start(out=stats_out[:], in_=stat_sb0[:])
                nc.sync.dma_start(out=idx_out[:],
                                  in_=idx_all[:].rearrange("p a b c -> p (a b c)"))
                nc.sync.dma_start(out=w_out[:],
                                  in_=w_all[:].rearrange("p a b c -> p (a b c)"))

            if variant == "v3":
                # one 2MB DMA per contraction chunk; 8 col-tiled matmuls
                # (2 halves x 4 col-groups) consume it; PSUM holds all
                # 4096 tokens' logitsT in 2 banks.
                NGCOL = 4
                ps_halves = [psL.tile([128, TG], F32, tag="psL",
                                      name=f"psLh_{h}") for h in range(2)]
                for c in range(NCHUNK):
                    xt = xpool.tile([128, TC], F32, tag="xt")
                    nc.sync.dma_start(out=xt[:],
                                      in_=xT[128 * c:128 * (c + 1), :])
                    for h in range(2):
                        for g in range(NGCOL):
                            nc.tensor.matmul(
                                ps_halves[h][32 * g:32 * g + E, :],
                                wt_sb[:, c, :],
                                xt[:, 2048 * h + TG * g:
                                   2048 * h + TG * (g + 1)],
                                start=(c == 0), stop=(c == NCHUNK - 1),
                                tile_position=(0, 32 * g))
                ps_sel_all = [ps_halves[h][32 * g:32 * g + E, :]
                              for h in range(2) for g in range(NGCOL)]

            NGCOL = 2 if variant in ("v7", "v8", "v9") else (4 if variant in ("v2", "v4", "v5", "v6") else NG)
            QV2 = TC // (TG * NGCOL)               # 2 halves for v2/v4
            if variant == "dma":
                outer = []
            elif variant == "v3":
                outer = ["all"]
            else:
                outer = list(range(QV2 if variant in ("v2", "v4", "v5", "v6", "v7", "v8", "v9") else NQ))

            for q in outer:
                if variant == "v3":
                    ps_sel = ps_sel_all
                    q = 0
                elif variant == "v6":
                    ps_half = psL.tile([128, TG], F32, tag="psL",
                                       name=f"psL_{q}")
                    TQV = TG * NGCOL               # 2048 tokens per half
                    for c in range(NCHUNK):
                        for part in range(2):
                            xt = xpool.tile([128, TQV // 2], F32, tag="xt")
                            eng = nc.sync if (2 * c + part) % 2 == 0 \
                                else nc.scalar
                            eng.dma_start(
                                out=xt[:],
                                in_=xT[128 * c:128 * (c + 1),
                                       TQV * q + 1024 * part:
                                       TQV * q + 1024 * (part + 1)])
                            for gg in range(2):
                                g = 2 * part + gg
                                nc.tensor.matmul(
                                    ps_half[32 * g:32 * g + E, :],
                                    wt_sb[:, c, :],
                                    xt[:, TG * gg:TG * (gg + 1)],
                                    start=(c == 0), stop=(c == NCHUNK - 1),
                                    tile_position=(0, 32 * g))
                    ps_sel = [ps_half[32 * g:32 * g + E, :]
                              for g in range(NGCOL)]
                elif variant in ("v2", "v4", "v5", "v7", "v8", "v9"):
                    ps_half = psL.tile([128, TG], F32, tag="psL",
                                       name=f"psL_{q}")
                    TQV = TG * NGCOL               # tokens per section
                    for c in range(NCHUNK):
                        xt = xpool.tile([128, TQV], F32, tag="xt")
                        if variant == "v8":
                            eng = [nc.sync, nc.scalar,
                                   nc.gpsimd][(q * NCHUNK + c) % 3]
                        else:
                            eng = (nc.scalar
                                   if (variant in ("v4", "v5", "v7", "v9")
                                       and c % 2)
                                   else nc.sync)
                        if variant == "v5":
                            eng.dma_start(
                                out=xt[:],
                                in_=xT[H * q + 128 * c:H * q + 128 * (c + 1),
                                       :])
                        else:
                            eng.dma_start(
                                out=xt[:],
                                in_=xT[128 * c:128 * (c + 1),
                                       TQV * q:TQV * (q + 1)])
                        for g in range(NGCOL):
                            nc.tensor.matmul(
                                ps_half[32 * g:32 * g + E, :],
                                wt_sb[:, c, :],
                                xt[:, TG * g:TG * (g + 1)],
                                start=(c == 0), stop=(c == NCHUNK - 1),
                                tile_position=(0, 32 * g))
                    ps_sel = [ps_half[32 * g:32 * g + E, :]
                              for g in range(NGCOL)]
                else:
                    ps_g = [psL.tile([E, TG], F32, tag="psL",
                                     name=f"psL_{q}_{g}")
                            for g in range(NG)]
                    for c in range(NCHUNK):
                        xt = xpool.tile([128, TQ], F32, tag="xt")
                        nc.sync.dma_start(
                            out=xt[:],
                            in_=xT[128 * c:128 * (c + 1), TQ * q:TQ * (q + 1)])
                        for g in range(NG):
                            nc.tensor.matmul(
                                ps_g[g][:],
                                wt_sb[:, c, :],
                                xt[:, TG * g:TG * (g + 1)],
                                start=(c == 0), stop=(c == NCHUNK - 1))
                    ps_sel = [ps_g[g][:] for g in range(NG)]

                for g in range(len(ps_sel)):
                    grp = q * len(ps_sel) + g
                    # logitsT [8, 512] -> SBUF
                    lg = lgpool.tile([E, TG], F32, tag="lg")
                    nc.scalar.copy(lg[:], ps_sel[g])
                    # transpose to token-major [128, 4, 8]
                    tp = psT.tile([128, NSUB * E], F32, tag="tp")
                    for j in range(NSUB):
                        nc.tensor.transpose(tp[:, E * j:E * (j + 1)],
                                            lg[:, 128 * j:128 * (j + 1)],
                                            eye_sb[:])
                    lt = ltpool.tile([128, NSUB, E], F32, tag="lt")
                    nc.vector.tensor_copy(lt[:], tp[:])

                    # top-8 sort + indices per token
                    mx = spool.tile([128, NSUB, E], F32, tag="mx")
                    mi = spool.tile([128, NSUB, E], U32, tag="mi")
                    for j in range(NSUB):
                        nc.vector.max(out=mx[:, j, :], in_=lt[:, j, :])
                        nc.vector.max_index(out=mi[:, j, :], in_max=mx[:, j, :],
                                            in_values=lt[:, j, :])

                    # softmax probs p = exp(l) / sum(exp(l))  (no shift:
                    # |logit| < ~7 so exp is safe in f32)
                    ea = spool.tile([128, NSUB, E], F32, tag="ea")
                    nc.scalar.activation(ea[:], lt[:], AF.Exp)
                    s4 = spool.tile([128, NSUB], F32, tag="s4")
                    nc.vector.tensor_reduce(s4[:], ea[:], AX.X, ALU.add)
                    r4 = spool.tile([128, NSUB], F32, tag="r4")
                    nc.vector.reciprocal(r4[:], s4[:])

                    # stats tile: [p(8) | 2hot(8)] per subtile
                    st = spool.tile([128, NSUB, 2 * E], F32, tag="st")
                    for j in range(NSUB):
                        nc.vector.scalar_tensor_tensor(
                            out=st[:, j, 0:E], in0=ea[:, j, :],
                            scalar=r4[:, j:j + 1], in1=ea[:, j, :],
                            op0=ALU.mult, op1=ALU.bypass)
                        nc.vector.tensor_tensor(
                            out=st[:, j, E:2 * E], in0=lt[:, j, :],
                            in1=mx[:, j, 1:2].to_broadcast([128, E]),
                            op=ALU.is_ge)
                    nc.tensor.matmul(stat_ps[:], st[:], ones_sb[:],
                                     start=(grp == 0), stop=(grp == NGRP - 1))

                    # top-2 normalized weights: e(top1), e(top2) / their sum
                    e2 = spool.tile([128, NSUB, K], F32, tag="e2")
                    nc.scalar.activation(e2[:], mx[:, :, 0:K], AF.Exp)
                    s2 = spool.tile([128, NSUB], F32, tag="s2")
                    nc.vector.tensor_reduce(s2[:], e2[:], AX.X, ALU.add)
                    r2 = spool.tile([128, NSUB], F32, tag="r2")
                    nc.vector.reciprocal(r2[:], s2[:])
                    for j in range(NSUB):
                        nc.vector.scalar_tensor_tensor(
                            out=w_all[:, grp, j, :], in0=e2[:, j, :],
                            scalar=r2[:, j:j + 1], in1=e2[:, j, :],
                            op0=ALU.mult, op1=ALU.bypass)
                    nc.vector.tensor_copy(idx_all[:, grp, :, :], mi[:, :, 0:K])

                if variant == "v9":
                    ng = len(ps_sel)
                    g0 = q * ng
                    nc.sync.dma_start(
                        out=idx_out[:, NSUB * K * g0:NSUB * K * (g0 + ng)],
                        in_=idx_all[:, g0:g0 + ng, :, :].rearrange(
                            "p a b c -> p (a b c)"))
                    nc.scalar.dma_start(
                        out=w_out[:, NSUB * K * g0:NSUB * K * (g0 + ng)],
                        in_=w_all[:, g0:g0 + ng, :, :].rearrange(
                            "p a b c -> p (a b c)"))

            if variant != "dma":
                loop_cm.__exit__(None, None, None)

            # --- drain outputs ---
            if variant != "dma":
                stat_sb = cpool.tile([2 * E * NSUB, 1], F32)
                nc.vector.tensor_copy(stat_sb[:], stat_ps[:])
                nc.sync.dma_start(out=stats_out[:], in_=stat_sb[:])
                if variant != "v9":
                    nc.sync.dma_start(
                        out=idx_out[:],
                        in_=idx_all[:].rearrange("p a b c -> p (a b c)"))
                    nc.sync.dma_start(
                        out=w_out[:],
                        in_=w_all[:].rearrange("p a b c -> p (a b c)"))

    nc.compile()
    return nc


def kernel(hidden_states, weight):
    global LAST_EXEC_NS, LAST_PROFILE
    from concourse.bass_utils import run_bass_kernel_spmd

    variant = os.environ.get("KMOE_VARIANT", "v7")
    if variant not in _CACHE:
        _CACHE[variant] = _build(variant)
    nc = _CACHE[variant]

    x = np.ascontiguousarray(np.asarray(hidden_states, dtype=np.float32)
                             .reshape(T, H))
    w = np.asarray(weight, dtype=np.float32)
    wTh = np.ascontiguousarray(w.T)                       # [H, E]
    eye = np.eye(E, dtype=np.float32)

    in_maps = []
    for i in range(NCORES):
        shard = x[TC * i:TC * (i + 1)]
        if variant == "v5":
            xTi = np.ascontiguousarray(
                np.concatenate([shard[:TC // 2].T, shard[TC // 2:].T]))
        else:
            xTi = np.ascontiguousarray(shard.T)               # [H, TC]
        in_maps.append({"xT": xTi, "wT": wTh, "eye": eye})

    trace = os.environ.get("KMOE_TRACE", "0") == "1"
    res = run_bass_kernel_spmd(nc, in_maps, list(range(NCORES)), trace=trace)
    LAST_EXEC_NS = res.exec_time_ns
    LAST_PROFILE = res.profile_json

    NSUB = TG // 128
    NGRP = NQ * NG
    topk_idx = np.empty((T, K), dtype=np.int32)
    topk_w = np.empty((T, K), dtype=np.float32)
    psum_core = np.empty((NCORES, E), dtype=np.float64)
    cnt_core = np.empty((NCORES, E), dtype=np.float64)
    for i in range(NCORES):
        r = res.results[i]
        ia = r["idx_out"].reshape(128, NGRP, NSUB, K)
        wa = r["w_out"].reshape(128, NGRP, NSUB, K)
        # token t = 512*grp + 128*j + p  ->  [grp, j, p, k]
        topk_idx[TC * i:TC * (i + 1)] = (
            ia.transpose(1, 2, 0, 3).reshape(TC, K))
        topk_w[TC * i:TC * (i + 1)] = (
            wa.transpose(1, 2, 0, 3).reshape(TC, K))
        stats = r["stats_out"].reshape(NSUB, 2 * E).astype(np.float64)
        psum_core[i] = stats[:, 0:E].sum(axis=0)
        cnt_core[i] = stats[:, E:2 * E].sum(axis=0)

    cores_per_b = NCORES // B
    pi = psum_core.reshape(B, cores_per_b, E).sum(axis=1) / S      # [B, E]
    ce = cnt_core.reshape(B, cores_per_b, E).sum(axis=1) / (S * K / E)
    aux = np.float32((ce * pi).sum(axis=1).mean() * ALPHA)
    return topk_idx, topk_w, aux


# revision 23
# speedup vs baseline: 1.0439x; 1.0439x over previous
"""MoE gate routing kernel for 8 Trainium2 NeuronCores.

Problem: hidden_states [4, 8192, 2048] f32, weight [8, 2048] f32.
  logits = x @ w.T ; scores = softmax(logits); top-2 (values+indices);
  normalized top-2 weights; seq-aux load-balancing loss scalar.

Sharding: 4096 tokens per core (token-parallel; each core's tokens lie in
one batch row). The gate weight is replicated. Per-core partial per-expert
score sums and top-2 counts are folded into the aux-loss scalar on host.

The device kernel consumes x pre-transposed ([H, T_core]) so the H
(contraction) dim lands on SBUF partitions with unit-efficiency DMAs.
All matmuls are exact fp32.
"""
import os
import numpy as np

B, S, H = 4, 8192, 2048
E, K = 8, 2
NCORES = 8
T = B * S                 # 32768 tokens
TC = T // NCORES          # 4096 tokens per core
NQ = 4                    # quarters per core
TQ = TC // NQ             # 1024 tokens per quarter
NG = 2                    # 512-token groups per quarter
TG = TQ // NG             # 512
NCHUNK = H // 128         # 16 contraction chunks
ALPHA = 0.1

_CACHE = {}
LAST_EXEC_NS = None
LAST_PROFILE = None


def _build(variant, loop_n=None):
    import contextlib
    import concourse.bacc as bacc
    import concourse.tile as tile
    from concourse import mybir

    F32 = mybir.dt.float32
    I32 = mybir.dt.int32
    U32 = mybir.dt.uint32
    AF = mybir.ActivationFunctionType
    ALU = mybir.AluOpType
    AX = mybir.AxisListType

    nc = bacc.Bacc("TRN2", target_bir_lowering=False, debug=False,
                   num_devices=NCORES)
    if variant == "v5":
        xT = nc.declare_dram_parameter("xT", [2 * H, TC // 2], F32,
                                       isOutput=False)
    else:
        xT = nc.declare_dram_parameter("xT", [H, TC], F32, isOutput=False)
    wT = nc.declare_dram_parameter("wT", [H, E], F32, isOutput=False)
    eye = nc.declare_dram_parameter("eye", [E, E], F32, isOutput=False)
    idx_out = nc.declare_dram_parameter("idx_out", [128, TC * K // 128], I32,
                                        isOutput=True)
    w_out = nc.declare_dram_parameter("w_out", [128, TC * K // 128], F32,
                                      isOutput=True)
    stats_out = nc.declare_dram_parameter("stats_out", [2 * E * (TG // 128), 1],
                                          F32, isOutput=True)

    NSUB = TG // 128            # 128-token subtiles per group = 4
    NGRP = NQ * NG              # 8 groups of 512 tokens per core

    with tile.TileContext(nc) as tc:
        with (
            tc.tile_pool(name="const", bufs=1) as cpool,
            tc.tile_pool(name="xt", bufs=12 if variant in ("v6", "v7", "v8", "v9", "v10") else (8 if variant in ("v4", "v5") else 6)) as xpool,
            tc.tile_pool(name="lg", bufs=4) as lgpool,
            tc.tile_pool(name="lt", bufs=2) as ltpool,
            tc.tile_pool(name="small", bufs=2) as spool,
            tc.tile_pool(name="outacc", bufs=1) as opool,
            tc.tile_pool(name="psL", bufs=4, space="PSUM") as psL,
            tc.tile_pool(name="psT", bufs=2, space="PSUM") as psT,
            tc.tile_pool(name="psS", bufs=1, space="PSUM") as psS,
        ):
            # --- constants ---
            wt_sb = cpool.tile([128, NCHUNK, E], F32)     # weight chunks
            for c in range(NCHUNK):
                nc.sync.dma_start(out=wt_sb[:, c, :],
                                  in_=wT[128 * c:128 * (c + 1), :])
            eye_sb = cpool.tile([E, E], F32)
            nc.sync.dma_start(out=eye_sb[:], in_=eye[:])
            ones_sb = cpool.tile([128, 1], F32)
            nc.vector.memset(ones_sb[:], 1.0)

            # --- persistent accumulators ---
            idx_all = opool.tile([128, NGRP, NSUB, K], I32)
            w_all = opool.tile([128, NGRP, NSUB, K], F32)
            stat_ps = psS.tile([2 * E * NSUB, 1], F32)

            # bench-only: repeat the whole body on-device to time the
            # steady-state kernel without per-execution dispatch overhead
            loop_cm = tc.For_i(0, loop_n, 1) if loop_n else contextlib.nullcontext()
            loop_cm.__enter__()

            if variant == "dma":
                # memory-floor probe: stream all of xT in, minimal consume
                acc = cpool.tile([128, 1], F32)
                for q in range(2):
                    for c in range(NCHUNK):
                        xt = xpool.tile([128, TC // 2], F32, tag="xt")
                        nc.sync.dma_start(
                            out=xt[:],
                            in_=xT[128 * c:128 * (c + 1),
                                   (TC // 2) * q:(TC // 2) * (q + 1)])
                        nc.vector.tensor_reduce(acc[:], xt[:, 0:8], AX.X,
                                                ALU.add)
                loop_cm.__exit__(None, None, None)
                nc.vector.memset(idx_all[:].rearrange("p a b c -> p (a b c)"), 0)
                nc.vector.memset(w_all[:].rearrange("p a b c -> p (a b c)"), 0.0)
                stat_sb0 = cpool.tile([2 * E * NSUB, 1], F32)
                nc.vector.memset(stat_sb0[:], 0.0)
                nc.sync.dma_start(out=stats_out[:], in_=stat_sb0[:])
                nc.sync.dma_start(out=idx_out[:],
                                  in_=idx_all[:].rearrange("p a b c -> p (a b c)"))
                nc.sync.dma_start(out=w_out[:],
                                  in_=w_all[:].rearrange("p a b c -> p (a b c)"))

            if variant == "v10":
                # v7's exact DMA structure (64 x 512KB, dual-ring, bufs=12)
                # with compute stripped: the true floor for v7's stream
                acc10 = cpool.tile([128, 1], F32)
                for q in range(4):
                    for c in range(NCHUNK):
                        xt = xpool.tile([128, TQ], F32, tag="xt")
                        eng = nc.scalar if c % 2 else nc.sync
                        eng.dma_start(
                            out=xt[:],
                            in_=xT[128 * c:128 * (c + 1),
                                   TQ * q:TQ * (q + 1)])
                        nc.vector.tensor_reduce(acc10[:], xt[:, 0:8], AX.X,
                                                ALU.add)
                loop_cm.__exit__(None, None, None)
                nc.vector.memset(idx_all[:].rearrange("p a b c -> p (a b c)"), 0)
                nc.vector.memset(w_all[:].rearrange("p a b c -> p (a b c)"), 0.0)
                st10 = cpool.tile([2 * E * NSUB, 1], F32)
                nc.vector.memset(st10[:], 0.0)
                nc.sync.dma_start(out=stats_out[:], in_=st10[:])
                nc.sync.dma_start(out=idx_out[:],
                                  in_=idx_all[:].rearrange("p a b c -> p (a b c)"))
                nc.sync.dma_start(out=w_out[:],
                                  in_=w_all[:].rearrange("p a b c -> p (a b c)"))

            if variant == "v3":
                # one 2MB DMA per contraction chunk; 8 col-tiled matmuls
                # (2 halves x 4 col-groups) consume it; PSUM holds all
                # 4096 tokens' logitsT in 2 banks.
                NGCOL = 4
                ps_halves = [psL.tile([128, TG], F32, tag="psL",
                                      name=f"psLh_{h}") for h in range(2)]
                for c in range(NCHUNK):
                    xt = xpool.tile([128, TC], F32, tag="xt")
                    nc.sync.dma_start(out=xt[:],
                                      in_=xT[128 * c:128 * (c + 1), :])
                    for h in range(2):
                        for g in range(NGCOL):
                            nc.tensor.matmul(
                                ps_halves[h][32 * g:32 * g + E, :],
                                wt_sb[:, c, :],
                                xt[:, 2048 * h + TG * g:
                                   2048 * h + TG * (g + 1)],
                                start=(c == 0), stop=(c == NCHUNK - 1),
                                tile_position=(0, 32 * g))
                ps_sel_all = [ps_halves[h][32 * g:32 * g + E, :]
                              for h in range(2) for g in range(NGCOL)]

            NGCOL = 2 if variant in ("v7", "v8", "v9") else (4 if variant in ("v2", "v4", "v5", "v6") else NG)
            QV2 = TC // (TG * NGCOL)               # 2 halves for v2/v4
            if variant in ("dma", "v10"):
                outer = []
            elif variant == "v3":
                outer = ["all"]
            else:
                outer = list(range(QV2 if variant in ("v2", "v4", "v5", "v6", "v7", "v8", "v9") else NQ))

            for q in outer:
                if variant == "v3":
                    ps_sel = ps_sel_all
                    q = 0
                elif variant == "v6":
                    ps_half = psL.tile([128, TG], F32, tag="psL",
                                       name=f"psL_{q}")
                    TQV = TG * NGCOL               # 2048 tokens per half
                    for c in range(NCHUNK):
                        for part in range(2):
                            xt = xpool.tile([128, TQV // 2], F32, tag="xt")
                            eng = nc.sync if (2 * c + part) % 2 == 0 \
                                else nc.scalar
                            eng.dma_start(
                                out=xt[:],
                                in_=xT[128 * c:128 * (c + 1),
                                       TQV * q + 1024 * part:
                                       TQV * q + 1024 * (part + 1)])
                            for gg in range(2):
                                g = 2 * part + gg
                                nc.tensor.matmul(
                                    ps_half[32 * g:32 * g + E, :],
                                    wt_sb[:, c, :],
                                    xt[:, TG * gg:TG * (gg + 1)],
                                    start=(c == 0), stop=(c == NCHUNK - 1),
                                    tile_position=(0, 32 * g))
                    ps_sel = [ps_half[32 * g:32 * g + E, :]
                              for g in range(NGCOL)]
                elif variant in ("v2", "v4", "v5", "v7", "v8", "v9"):
                    ps_half = psL.tile([128, TG], F32, tag="psL",
                                       name=f"psL_{q}")
                    TQV = TG * NGCOL               # tokens per section
                    for c in range(NCHUNK):
                        xt = xpool.tile([128, TQV], F32, tag="xt")
                        if variant == "v8":
                            eng = [nc.sync, nc.scalar,
                                   nc.gpsimd][(q * NCHUNK + c) % 3]
                        else:
                            eng = (nc.scalar
                                   if (variant in ("v4", "v5", "v7", "v9")
                                       and c % 2)
                                   else nc.sync)
                        if variant == "v5":
                            eng.dma_start(
                                out=xt[:],
                                in_=xT[H * q + 128 * c:H * q + 128 * (c + 1),
                                       :])
                        else:
                            eng.dma_start(
                                out=xt[:],
                                in_=xT[128 * c:128 * (c + 1),
                                       TQV * q:TQV * (q + 1)])
                        for g in range(NGCOL):
                            nc.tensor.matmul(
                                ps_half[32 * g:32 * g + E, :],
                                wt_sb[:, c, :],
                                xt[:, TG * g:TG * (g + 1)],
                                start=(c == 0), stop=(c == NCHUNK - 1),
                                tile_position=(0, 32 * g))
                    ps_sel = [ps_half[32 * g:32 * g + E, :]
                              for g in range(NGCOL)]
                else:
                    ps_g = [psL.tile([E, TG], F32, tag="psL",
                                     name=f"psL_{q}_{g}")
                            for g in range(NG)]
                    for c in range(NCHUNK):
                        xt = xpool.tile([128, TQ], F32, tag="xt")
                        nc.sync.dma_start(
                            out=xt[:],
                            in_=xT[128 * c:128 * (c + 1), TQ * q:TQ * (q + 1)])
                        for g in range(NG):
                            nc.tensor.matmul(
                                ps_g[g][:],
                                wt_sb[:, c, :],
                                xt[:, TG * g:TG * (g + 1)],
                                start=(c == 0), stop=(c == NCHUNK - 1))
                    ps_sel = [ps_g[g][:] for g in range(NG)]

                for g in range(len(ps_sel)):
                    grp = q * len(ps_sel) + g
                    # logitsT [8, 512] -> SBUF
                    lg = lgpool.tile([E, TG], F32, tag="lg")
                    nc.scalar.copy(lg[:], ps_sel[g])
                    # transpose to token-major [128, 4, 8]
                    tp = psT.tile([128, NSUB * E], F32, tag="tp")
                    for j in range(NSUB):
                        nc.tensor.transpose(tp[:, E * j:E * (j + 1)],
                                            lg[:, 128 * j:128 * (j + 1)],
                                            eye_sb[:])
                    lt = ltpool.tile([128, NSUB, E], F32, tag="lt")
                    nc.vector.tensor_copy(lt[:], tp[:])

                    # top-8 sort + indices per token
                    mx = spool.tile([128, NSUB, E], F32, tag="mx")
                    mi = spool.tile([128, NSUB, E], U32, tag="mi")
                    for j in range(NSUB):
                        nc.vector.max(out=mx[:, j, :], in_=lt[:, j, :])
                        nc.vector.max_index(out=mi[:, j, :], in_max=mx[:, j, :],
                                            in_values=lt[:, j, :])

                    # softmax probs p = exp(l) / sum(exp(l))  (no shift:
                    # |logit| < ~7 so exp is safe in f32)
                    ea = spool.tile([128, NSUB, E], F32, tag="ea")
                    nc.scalar.activation(ea[:], lt[:], AF.Exp)
                    s4 = spool.tile([128, NSUB], F32, tag="s4")
                    nc.vector.tensor_reduce(s4[:], ea[:], AX.X, ALU.add)
                    r4 = spool.tile([128, NSUB], F32, tag="r4")
                    nc.vector.reciprocal(r4[:], s4[:])

                    # stats tile: [p(8) | 2hot(8)] per subtile
                    st = spool.tile([128, NSUB, 2 * E], F32, tag="st")
                    for j in range(NSUB):
                        nc.vector.scalar_tensor_tensor(
                            out=st[:, j, 0:E], in0=ea[:, j, :],
                            scalar=r4[:, j:j + 1], in1=ea[:, j, :],
                            op0=ALU.mult, op1=ALU.bypass)
                        nc.vector.tensor_tensor(
                            out=st[:, j, E:2 * E], in0=lt[:, j, :],
                            in1=mx[:, j, 1:2].to_broadcast([128, E]),
                            op=ALU.is_ge)
                    nc.tensor.matmul(stat_ps[:], st[:], ones_sb[:],
                                     start=(grp == 0), stop=(grp == NGRP - 1))

                    # top-2 normalized weights: e(top1), e(top2) / their sum
                    e2 = spool.tile([128, NSUB, K], F32, tag="e2")
                    nc.scalar.activation(e2[:], mx[:, :, 0:K], AF.Exp)
                    s2 = spool.tile([128, NSUB], F32, tag="s2")
                    nc.vector.tensor_reduce(s2[:], e2[:], AX.X, ALU.add)
                    r2 = spool.tile([128, NSUB], F32, tag="r2")
                    nc.vector.reciprocal(r2[:], s2[:])
                    for j in range(NSUB):
                        nc.vector.scalar_tensor_tensor(
                            out=w_all[:, grp, j, :], in0=e2[:, j, :],
                            scalar=r2[:, j:j + 1], in1=e2[:, j, :],
                            op0=ALU.mult, op1=ALU.bypass)
                    nc.vector.tensor_copy(idx_all[:, grp, :, :], mi[:, :, 0:K])

                if variant == "v9":
                    ng = len(ps_sel)
                    g0 = q * ng
                    nc.sync.dma_start(
                        out=idx_out[:, NSUB * K * g0:NSUB * K * (g0 + ng)],
                        in_=idx_all[:, g0:g0 + ng, :, :].rearrange(
                            "p a b c -> p (a b c)"))
                    nc.scalar.dma_start(
                        out=w_out[:, NSUB * K * g0:NSUB * K * (g0 + ng)],
                        in_=w_all[:, g0:g0 + ng, :, :].rearrange(
                            "p a b c -> p (a b c)"))

            if variant not in ("dma", "v10"):
                loop_cm.__exit__(None, None, None)

            # --- drain outputs ---
            if variant not in ("dma", "v10"):
                stat_sb = cpool.tile([2 * E * NSUB, 1], F32)
                nc.vector.tensor_copy(stat_sb[:], stat_ps[:])
                nc.sync.dma_start(out=stats_out[:], in_=stat_sb[:])
                if variant != "v9":
                    nc.sync.dma_start(
                        out=idx_out[:],
                        in_=idx_all[:].rearrange("p a b c -> p (a b c)"))
                    nc.sync.dma_start(
                        out=w_out[:],
                        in_=w_all[:].rearrange("p a b c -> p (a b c)"))

    nc.compile()
    return nc


def kernel(hidden_states, weight):
    global LAST_EXEC_NS, LAST_PROFILE
    from concourse.bass_utils import run_bass_kernel_spmd

    variant = os.environ.get("KMOE_VARIANT", "v7")
    if variant not in _CACHE:
        _CACHE[variant] = _build(variant)
    nc = _CACHE[variant]

    x = np.ascontiguousarray(np.asarray(hidden_states, dtype=np.float32)
                             .reshape(T, H))
    w = np.asarray(weight, dtype=np.float32)
    wTh = np.ascontiguousarray(w.T)                       # [H, E]
    eye = np.eye(E, dtype=np.float32)

    in_maps = []
    for i in range(NCORES):
        shard = x[TC * i:TC * (i + 1)]
        if variant == "v5":
            xTi = np.ascontiguousarray(
                np.concatenate([shard[:TC // 2].T, shard[TC // 2:].T]))
        else:
            xTi = np.ascontiguousarray(shard.T)               # [H, TC]
        in_maps.append({"xT": xTi, "wT": wTh, "eye": eye})

    trace = os.environ.get("KMOE_TRACE", "0") == "1"
    res = run_bass_kernel_spmd(nc, in_maps, list(range(NCORES)), trace=trace)
    LAST_EXEC_NS = res.exec_time_ns
    LAST_PROFILE = res.profile_json

    NSUB = TG // 128
    NGRP = NQ * NG
    topk_idx = np.empty((T, K), dtype=np.int32)
    topk_w = np.empty((T, K), dtype=np.float32)
    psum_core = np.empty((NCORES, E), dtype=np.float64)
    cnt_core = np.empty((NCORES, E), dtype=np.float64)
    for i in range(NCORES):
        r = res.results[i]
        ia = r["idx_out"].reshape(128, NGRP, NSUB, K)
        wa = r["w_out"].reshape(128, NGRP, NSUB, K)
        # token t = 512*grp + 128*j + p  ->  [grp, j, p, k]
        topk_idx[TC * i:TC * (i + 1)] = (
            ia.transpose(1, 2, 0, 3).reshape(TC, K))
        topk_w[TC * i:TC * (i + 1)] = (
            wa.transpose(1, 2, 0, 3).reshape(TC, K))
        stats = r["stats_out"].reshape(NSUB, 2 * E).astype(np.float64)
        psum_core[i] = stats[:, 0:E].sum(axis=0)
        cnt_core[i] = stats[:, E:2 * E].sum(axis=0)

    cores_per_b = NCORES // B
    pi = psum_core.reshape(B, cores_per_b, E).sum(axis=1) / S      # [B, E]
    ce = cnt_core.reshape(B, cores_per_b, E).sum(axis=1) / (S * K / E)
    aux = np.float32((ce * pi).sum(axis=1).mean() * ALPHA)
    return topk_idx, topk_w, aux
